# revision 11
# baseline (speedup 1.0000x reference)
"""Trainium2 Bass kernel for column-softmax attention.

reference semantics:
    scores = einsum('bqd,bkd->bqk', q, k) / sqrt(128)   # [B, Nq, Nk]
    attn   = softmax(scores, axis=1)                     # over the QUERY axis
    out    = einsum('bqk,bkd->bqd', attn, v)             # [B, Nq, D]

Because the softmax is over q, each key column k normalizes independently:
    out[q, d] = sum_k E[k, q] * r[k] * v[k, d],  E = exp(scores.T), r = 1/sum_q E[k, q]

Sharding: 8 cores = 4 batches x 2 key-halves.  Each core computes the partial
sum over its 2048 keys; the host adds the two partials per batch.

On-chip layout: the host pre-transposes Q and K to [D, N] (contraction dim on
partitions) and the kernel emits out.T [D, Nq]; the host transposes back.  The
softmax denominator is folded into V row-scaling so the normalize step touches
only 128x128 tiles.
"""

import numpy as np

import concourse.bass as bass
import concourse.mybir as mybir
import concourse.tile as tile
from concourse.bass_utils import run_bass_kernel_spmd

B, N, D = 4, 4096, 128
P = 128
NK = 2048                 # keys per core (half of 4096)
KT_TILES = NK // P        # 16 key tiles of 128
GROUP = 4                 # key tiles per PSUM-accumulation group
N_GROUPS = KT_TILES // GROUP
SCALE = 1.0 / np.sqrt(128.0)

F32 = mybir.dt.float32
F32R = mybir.dt.float32r
F16 = mybir.dt.float16

# exp(x - EXP_BIAS) rescales E down and the normalizer up by the same factor,
# keeping the result identical while centering vsc = v/sum(E) in fp16 range.
EXP_BIAS = float(np.log(4096.0))


def build_bass():
    nc = bass.Bass("TRN2", target_bir_lowering=False, debug=False)
    qt_d = nc.dram_tensor("qt", [P, N], F16, kind="ExternalInput").ap()
    kt_d = nc.dram_tensor("kt", [P, NK], F16, kind="ExternalInput").ap()
    v_d = nc.dram_tensor("v", [NK, D], F32, kind="ExternalInput").ap()
    out_d = nc.dram_tensor("out_t", [P, N], F32, kind="ExternalOutput").ap()

    with tile.TileContext(nc) as tc:
        with (
            tc.tile_pool(name="big", bufs=1) as big,
            tc.tile_pool(name="epool", bufs=5) as epool,
            tc.tile_pool(name="small", bufs=8) as small,
            tc.tile_pool(name="spsum", bufs=3, space="PSUM") as spsum,
            tc.tile_pool(name="opsum", bufs=2, space="PSUM") as opsum,
        ):
            qT = big.tile([P, N], F16)          # [d, q]
            kT = big.tile([P, NK], F16)          # [d, k]
            vsb = big.tile([P, KT_TILES, D], F32)  # [k_in_tile, k_tile, d]
            oacc = big.tile([P, N], F32)        # [d, q] accumulator

            nc.sync.dma_start(qT[:], qt_d[:])
            nc.sync.dma_start(kT[:], kt_d[:])
            nc.sync.dma_start(vsb[:], v_d.rearrange("(t p) d -> p t d", p=P))


            # Warm-up matmul: absorbs the input-DMA waits so the first real
            # matmul carries at most one sync wait (hw limit for fp32r).
            Swarm = spsum.tile([P, 1024], F32, tag="S")
            nc.tensor.matmul(
                Swarm[0:1, 0:1], lhsT=kT[:, 0:1], rhs=qT[:, 0:1],
                start=True, stop=True,
            )

            for g in range(N_GROUPS):
                e_tiles = []
                v_tiles = []
                for j in range(GROUP):
                    ktile = g * GROUP + j
                    E = epool.tile([P, N], F16, tag="E")  # [k, q] = exp(scores.T)
                    rs = small.tile([P, 4], F32, tag=f"rs{ktile}")
                    for t in range(4):
                        S = spsum.tile([P, 1024], F32, tag="S")
                        for u in range(2):
                            nc.tensor.matmul(
                                S[:, u * 512 : (u + 1) * 512],
                                lhsT=kT[:, ktile * P : (ktile + 1) * P],
                                rhs=qT[:, t * 1024 + u * 512 : t * 1024 + (u + 1) * 512],
                                start=True,
                                stop=True,
                            )
                        nc.scalar.activation(
                            out=E[:, t * 1024 : (t + 1) * 1024],
                            in_=S[:],
                            func=mybir.ActivationFunctionType.Exp,
                            scale=float(SCALE),
                            accum_out=rs[:, t : t + 1],
                        )
                    rsum = small.tile([P, 1], F32, tag="rsum")
                    nc.vector.reduce_sum(out=rsum[:], in_=rs[:], axis=mybir.AxisListType.X)
                    recip = small.tile([P, 1], F32, tag="recip")
                    nc.vector.reciprocal(recip[:], rsum[:])
                    vsc = small.tile([P, D], F16, tag="vsc")  # [k, d] * r[k]
                    nc.vector.tensor_scalar_mul(vsc[:], vsb[:, ktile, :], recip[:])
                    e_tiles.append(E)
                    v_tiles.append(vsc)

                for c in range(N // 512):
                    O = opsum.tile([P, 512], F32, tag="O")  # [d, q-chunk]
                    for j in range(GROUP):
                        nc.tensor.matmul(
                            O[:],
                            lhsT=v_tiles[j][:],
                            rhs=e_tiles[j][:, c * 512 : (c + 1) * 512],
                            start=(j == 0),
                            stop=(j == GROUP - 1),
                        )
                    dst = oacc[:, c * 512 : (c + 1) * 512]
                    if g == 0:
                        nc.vector.tensor_copy(out=dst, in_=O[:])
                    else:
                        nc.vector.tensor_tensor(dst, dst, O[:], mybir.AluOpType.add)

            nc.sync.dma_start(out_d[:], oacc[:])
    return nc


def legalize_waits(nc, max_waits=1):
    """Hoist excess semaphore waits into standalone EventSemaphore ops.

    The walrus codegen for several engine instruction structs accepts only a
    single sync-wait command; Tile sometimes emits more.  Executing the extra
    waits in a preceding same-engine EventSemaphore is semantically identical
    (the engine runs its stream in order).
    """
    for fn in nc.m.functions:
        for blk in fn.blocks:
            out = []
            for inst in blk.instructions:
                si = inst.sync_info
                if (
                    si is not None
                    and si.on_wait
                    and len(si.on_wait) > max_waits
                    and inst.opcode != "EventSemaphore"
                ):
                    waits = list(si.on_wait)
                    extra, keep = waits[:-max_waits], waits[-max_waits:]
                    for n, w in enumerate(extra):
                        out.append(
                            mybir.InstEventSemaphore(
                                name=f"{inst.name}_prewait{n}",
                                engine=inst.engine,
                                ins=[],
                                outs=[],
                                sync_info=mybir.SyncInfo(on_wait=[w], on_update=[]),
                            )
                        )
                    si.on_wait = keep
                out.append(inst)
            blk.instructions = out
    return nc


_NC_CACHE = {}


def _get_nc():
    if "nc" not in _NC_CACHE:
        _NC_CACHE["nc"] = legalize_waits(build_bass())
    return _NC_CACHE["nc"]


def kernel(q, k, v):
    q = np.asarray(q, dtype=np.float32)
    k = np.asarray(k, dtype=np.float32)
    v = np.asarray(v, dtype=np.float32)

    in_maps = []
    for c in range(8):
        b, h = c // 2, c % 2
        in_maps.append(
            {
                "qt": np.ascontiguousarray(q[b].T).astype(np.float16),
                "kt": np.ascontiguousarray(k[b, h * NK : (h + 1) * NK].T).astype(np.float16),
                "v": np.ascontiguousarray(v[b, h * NK : (h + 1) * NK]),
            }
        )

    nc = _get_nc()
    res = run_bass_kernel_spmd(nc, in_maps, list(range(8))).results

    out = np.empty((B, N, D), dtype=np.float32)
    for b in range(B):
        out[b] = (res[2 * b]["out_t"] + res[2 * b + 1]["out_t"]).T
    return out
